# revision 51
# baseline (speedup 1.0000x reference)
"""DiffAttnV2-like fused kernel for Trainium2 (8 NeuronCores), v3.

Sharding: core = 4*b + g  (b = batch 0..1, g = head-group 0..3, 4 heads each).
Each core computes its 4 output heads' attention and a partial out = y_g @ Wo_g;
host sums the 4 partials per batch.

v3 changes vs the 587us v2 baseline:
  - softmax denominator moved off the PE: exp tiles are accumulated on the
    DVE (f16, 2x mode), partition-folded 128->16 with 3 in-place DVE adds,
    reciprocal'd at [16,512], then broadcast back to 128 partitions with a
    single ones16 PE matmul (213ns) -> the 139k-cycle ones-matmuls AND the
    16k den-broadcast matmuls disappear for ~23k new PE cycles.
    (gpsimd partition_all_reduce was tried first: 3.6us per call of Pool
    ucode, and it serialized behind the affine_selects - reverted.)
  - fp16 storage end-to-end instead of bf16 (same PE rate, 4x less
    quantization error, more slack for the f16 den accumulate)
  - output partials written as f16 (halves output DMA)
  - ppy PSUM pool bufs=3 (bank freed by the dead den accumulator)
  - phase-0 wq DMA split in dc-halves and issued before x so the first
    projection matmul starts ~6us earlier
"""
import sys
sys.path.insert(0, "/opt/trn_rl_repo")
from contextlib import ExitStack

import numpy as np

from concourse import bacc, mybir, tile
from concourse.bass_utils import run_bass_kernel_spmd

B, T, D, H = 2, 2048, 2048, 16
HPC = 4               # heads per core
NC = 8                # cores
NDC = D // 128        # 16 contraction chunks
NPH = 4               # t-phases
PT = T // NPH         # 512 t-cols per phase
SCALE = 1.0 / float(np.sqrt(D // H))

f32 = mybir.dt.float32
f32r = mybir.dt.float32r
f16 = mybir.dt.float16
EXP = mybir.ActivationFunctionType.Exp
GE = mybir.AluOpType.is_ge
ADD = mybir.AluOpType.add
MULT = mybir.AluOpType.mult
SUB = mybir.AluOpType.subtract

_CACHE = {}


def _build():
    nc = bacc.Bacc("TRN2", target_bir_lowering=False, debug=False)
    xTp = nc.dram_tensor("xTp", [NPH, 128, NDC, PT], f16, kind="ExternalInput").ap()
    wqp = nc.dram_tensor("wqp", [4, 128, NDC, 256], f16, kind="ExternalInput").ap()
    wkp = nc.dram_tensor("wkp", [2, 128, NDC, 256], f16, kind="ExternalInput").ap()
    wvp = nc.dram_tensor("wvp", [2, 128, NDC, 260], f16, kind="ExternalInput").ap()
    wop = nc.dram_tensor("wop", [4, 128, HPC, 512], f16, kind="ExternalInput").ap()
    sel16in = nc.dram_tensor("sel16in", [16, 2048], f16, kind="ExternalInput").ap()
    idenin = nc.dram_tensor("idenin", [128, 128], f32, kind="ExternalInput").ap()
    out = nc.dram_tensor("out", [T, D], f16, kind="ExternalOutput").ap()

    with tile.TileContext(nc) as tc, ExitStack() as ctx:
        ctx.enter_context(nc.allow_low_precision(reason="f16/fp32r pipeline"))
        persist = ctx.enter_context(tc.tile_pool(name="persist", bufs=1))
        xpool = ctx.enter_context(tc.tile_pool(name="xpool", bufs=2))
        qpool = ctx.enter_context(tc.tile_pool(name="qpool", bufs=2))
        wpool = ctx.enter_context(tc.tile_pool(name="wpool", bufs=3))
        wvpool = ctx.enter_context(tc.tile_pool(name="wvpool", bufs=2))
        epool = ctx.enter_context(tc.tile_pool(name="epool", bufs=3))
        apool = ctx.enter_context(tc.tile_pool(name="apool", bufs=4))
        dpool = ctx.enter_context(tc.tile_pool(name="dpool", bufs=2))
        cpool = ctx.enter_context(tc.tile_pool(name="cpool", bufs=2))
        ypool = ctx.enter_context(tc.tile_pool(name="ypool", bufs=2))
        opool = ctx.enter_context(tc.tile_pool(name="opool", bufs=4))
        # PSUM banks: pps s2[128,2,512]x2 = 4, ppy [128,512]x3 = 3, pptr = 1
        pps = ctx.enter_context(tc.tile_pool(name="pps", bufs=2, space="PSUM"))
        ppy = ctx.enter_context(tc.tile_pool(name="ppy", bufs=3, space="PSUM"))
        pptr = ctx.enter_context(tc.tile_pool(name="pptr", bufs=1, space="PSUM"))

        kT = persist.tile([128, HPC, T], f16)             # 16KB
        vn = persist.tile([128, 2, NDC, 2, 128], f16)     # 16KB [tk,(pair,tkc,pj),d]
        sel16 = persist.tile([16, 16, 128], f16)          # lam-row selectors
        iden = persist.tile([128, 128], f32)              # transpose identity
        ones128 = persist.tile([128, 128], f16)           # den reduce+bcast
        nc.vector.memset(ones128[:], 1.0)

        def emit_wo(phw, yhw, last=False):
            t0w = PT * phw
            for dout in range(4):
                wo4 = wvpool.tile([128, HPC, 512], f16, name=f"wo{phw}_{dout}",
                                  tag="wo4")
                nc.sync.dma_start(out=wo4[:], in_=wop[dout])
                for tsub in range(4):
                    alt = (dout * 4 + tsub) % 4
                    if alt < 3:
                        ps_o = ppy.tile([128, 512], f32,
                                        name=f"pso{phw}_{dout}_{tsub}", tag="y")
                    else:
                        ps_o = pptr.tile([128, 512], f32,
                                         name=f"pso{phw}_{dout}_{tsub}", tag="tr")
                    for hl in range(HPC):
                        nc.tensor.matmul(
                            ps_o[:], yhw[:, hl, 128 * tsub:128 * (tsub + 1)],
                            wo4[:, hl], start=(hl == 0), stop=(hl == HPC - 1))
                    ob = opool.tile([128, 512], f16,
                                    name=f"ob{phw}_{dout}_{tsub}", tag="ob")
                    vec_mod = 2 if last else 4
                    if (dout * 4 + tsub) % vec_mod == vec_mod - 1:
                        nc.vector.tensor_copy(ob[:], ps_o[:])
                    else:
                        nc.scalar.copy(ob[:], ps_o[:])
                    nc.sync.dma_start(
                        out=out[t0w + 128 * tsub:t0w + 128 * (tsub + 1),
                                512 * dout:512 * (dout + 1)],
                        in_=ob[:])

        prev_wo = None
        for ph in range(NPH):
            t0 = PT * ph
            # ---- phase x^T slice + q weights (ph0: wq halves first for
            #      a fast PE start) ----
            wq_tiles = []
            if ph == 0:
                # fine-grained first loads so the very first matmul chain
                # starts after ~0.4MB instead of ~2MB
                wtA1 = wpool.tile([128, 2, 256], f16, name="wq0_0A1",
                                  tag="wqA1", bufs=1)
                nc.sync.dma_start(out=wtA1[:], in_=wqp[0][:, 0:2])
                xThA1 = xpool.tile([128, 2, PT], f16, name="xThA1_0",
                                   tag="xThA1", bufs=1)
                nc.sync.dma_start(out=xThA1[:], in_=xTp[ph, :, 0:2])
                wtA2 = wpool.tile([128, NDC // 2 - 2, 256], f16, name="wq0_0A2",
                                  tag="wqA2", bufs=1)
                nc.sync.dma_start(out=wtA2[:], in_=wqp[0][:, 2:NDC // 2])
                xThA = xpool.tile([128, NDC // 2 - 2, PT], f16, name=f"xThA{ph}",
                                  tag="xThA", bufs=1)
                nc.sync.dma_start(out=xThA[:], in_=xTp[ph, :, 2:NDC // 2])
                wtB = wpool.tile([128, NDC // 2, 256], f16, name="wq0_0B",
                                 tag="wqB", bufs=1)
                nc.sync.dma_start(out=wtB[:], in_=wqp[0][:, NDC // 2:NDC])
                wq_tiles.append((wtA1, wtA2, wtB))
                xThB = xpool.tile([128, NDC // 2, PT], f16, name=f"xThB{ph}",
                                  tag="xThB")
                nc.sync.dma_start(out=xThB[:], in_=xTp[ph, :, NDC // 2:NDC])

                def xTh_(dc):
                    if dc < 2:
                        return xThA1[:, dc]
                    if dc < NDC // 2:
                        return xThA[:, dc - 2]
                    return xThB[:, dc - NDC // 2]
            else:
                xThA = xpool.tile([128, NDC // 2, PT], f16, name=f"xThA{ph}",
                                  tag="xThAn")
                nc.sync.dma_start(out=xThA[:], in_=xTp[ph, :, 0:NDC // 2])
                xThB = xpool.tile([128, NDC // 2, PT], f16, name=f"xThB{ph}",
                                  tag="xThB")
                nc.sync.dma_start(out=xThB[:], in_=xTp[ph, :, NDC // 2:NDC])

                def xTh_(dc):
                    return (xThA[:, dc] if dc < NDC // 2
                            else xThB[:, dc - NDC // 2])

            # ---- q projections (4 head-pairs) ----
            qTh = qpool.tile([128, 8, PT], f16, name=f"qTh{ph}", tag="qTh")
            ctx_q = nc.named_scope(f"proj{ph}"); ctx_q.__enter__()
            for pq in (0, 2, 1, 3):  # attention heads 0/1 need slots 0,1,4,5
                if ph == 0 and pq == 0:
                    wtA1, wtA2, wtB = wq_tiles[0]

                    def wq_(dc, wtA1=wtA1, wtA2=wtA2, wtB=wtB):
                        if dc < 2:
                            return wtA1[:, dc]
                        if dc < NDC // 2:
                            return wtA2[:, dc - 2]
                        return wtB[:, dc - NDC // 2]
                else:
                    wt = wpool.tile([128, NDC, 256], f16, name=f"wq{ph}_{pq}",
                                    tag="wq")
                    nc.sync.dma_start(out=wt[:], in_=wqp[pq])

                    def wq_(dc, wt=wt):
                        return wt[:, dc]
                ps = pps.tile([128, 2, PT], f32, name=f"psq{ph}_{pq}", tag="s2")
                for j in range(2):
                    for dc in range(NDC):
                        nc.tensor.matmul(ps[:, j],
                                         wq_(dc)[:, 128 * j:128 * (j + 1)],
                                         xTh_(dc),
                                         start=(dc == 0), stop=(dc == NDC - 1))
                nc.scalar.copy(qTh[:, 2 * pq:2 * pq + 2], ps[:])

            # ---- k projections (2 pairs) ----
            for pk in range(2):
                wt = wpool.tile([128, NDC, 256], f16, name=f"wk{ph}_{pk}",
                                tag="wq")
                nc.sync.dma_start(out=wt[:], in_=wkp[pk])
                ps = pps.tile([128, 2, PT], f32, name=f"psk{ph}_{pk}", tag="s2")
                for j in range(2):
                    for dc in range(NDC):
                        nc.tensor.matmul(ps[:, j], wt[:, dc, 128 * j:128 * (j + 1)],
                                         xTh_(dc),
                                         start=(dc == 0), stop=(dc == NDC - 1))
                nc.vector.tensor_copy(kT[:, 2 * pk:2 * pk + 2, t0:t0 + PT], ps[:])

            if ph == 0:  # selector DMAs behind the critical weight loads
                nc.sync.dma_start(out=sel16.rearrange("p a b -> p (a b)"),
                                  in_=sel16in[:])
                nc.sync.dma_start(out=iden[:], in_=idenin[:])

            # ---- v projections (natural [tk, d]); Wlam's 4 cols ride along
            #      pair 0's moving weights producing z = x@Wlam in natural
            #      layout ----
            lamz = cpool.tile([128, 2, 2, HPC], f32, name=f"lamz{ph}",
                              tag="lamz", bufs=1)
            for pair in range(2):
                nvc = 260 if pair == 0 else 256
                wt = wvpool.tile([128, NDC, 260], f16, name=f"wv{ph}_{pair}",
                                 tag="wv")
                nc.sync.dma_start(out=wt[:], in_=wvp[pair])
                for tg in range(2):  # tsub groups of 2
                    ps = pps.tile([128, 2, PT], f32, name=f"psv{ph}_{pair}_{tg}",
                                  tag="s2")
                    for t in range(2):
                        tsub = 2 * tg + t
                        for dc in range(NDC):
                            nc.tensor.matmul(
                                ps[:, t, 0:nvc],
                                xTh_(dc)[:, 128 * tsub:128 * (tsub + 1)],
                                wt[:, dc, 0:nvc],
                                start=(dc == 0), stop=(dc == NDC - 1))
                    if pair == 0:
                        nc.scalar.copy(
                            vn[:, pair, 4 * ph + 2 * tg:4 * ph + 2 * tg + 2],
                            ps[:, :, 0:256])
                        nc.scalar.copy(lamz[:, tg], ps[:, :, 256:260])
                    else:
                        nc.vector.tensor_copy(
                            vn[:, pair, 4 * ph + 2 * tg:4 * ph + 2 * tg + 2],
                            ps[:, :, 0:256])

            # lam = sigmoid(z) chain, emitted later (inside attention at
            # hl==0) so its scalar/DVE ops don't delay the first exps
            lam_state = {}

            def lam_finalize(lamz=lamz, ph=ph):
                lamE = cpool.tile([128, 16], f32, name=f"lamE{ph}", tag="lamE",
                                  bufs=1)
                nc.scalar.activation(lamE[:],
                                     lamz.rearrange("p a b c -> p (a b c)"),
                                     EXP, scale=-1.0)
                nc.vector.tensor_scalar_add(lamE[:], lamE[:], 1.0)
                lamF = cpool.tile([128, 16], f32, name=f"lamF{ph}", tag="lamF",
                                  bufs=1)
                nc.vector.reciprocal_approx_fast(lamF[:], lamE[:])
                psT = pptr.tile([128, PT], f32, name=f"psT{ph}", tag="tr")
                nc.tensor.transpose(psT[0:16, 0:128], lamF[:], iden[:])
                lamT = cpool.tile([16, 128], f16, name=f"lamT{ph}",
                                  tag="lamT", bufs=1)
                nc.scalar.copy(lamT[:], psT[0:16, 0:128])
                lam_state["lamT"] = lamT

            ctx_q.__exit__(None, None, None)
            # ---- Wo of previous phase (fills proj-evac stall window) ----
            if prev_wo is not None:
                with nc.named_scope(f"wo{ph-1}"):
                    emit_wo(*prev_wo)

            # ---- attention: 4 head-pairs ----
            ntk = 4 * (ph + 1)
            nbt = ntk // 2
            yh = ypool.tile([128, HPC, PT], f16, name=f"yh{ph}", tag="yh")
            pending_combine = None
            ctx_a = nc.named_scope(f"attn{ph}"); ctx_a.__enter__()
            for hl in range(HPC):
                meta = []
                for j, qh in ((0, hl), (1, 4 + hl)):
                    khl = 2 * (hl // 2) + j  # k/v slot (host packs [k,k'] pairs)
                    meta.append((qh, khl, khl // 2, khl % 2))
                ps_y = [ppy.tile([128, PT], f32, name=f"psy{ph}_{hl}_{j}",
                                 tag="y") for j in range(2)]
                acc = [apool.tile([128, 2, PT], f16, name=f"acc{ph}_{hl}_{j}",
                                  tag="acc") for j in range(2)]

                def consume(bt, exs):
                    for j in range(2):
                        _, _, pair, pj = meta[j]
                        for cc in range(2):
                            tkc = 2 * bt + cc
                            o = max(0, 128 * tkc - t0)
                            nc.tensor.matmul(ps_y[j][:, o:PT],
                                             vn[:, pair, tkc, pj],
                                             exs[j][:, cc, o:PT],
                                             start=(tkc == 0), stop=(tkc == ntk - 1))

                prev_full = [None, None]

                def acc_in(bt, j, ex):
                    """Fold the exp tile for (bt, j) into acc[j] on the DVE."""
                    a = acc[j]
                    exf = ex.rearrange("p a b -> p (a b)")
                    af = a.rearrange("p a b -> p (a b)")
                    if 2 * bt + 1 < 4 * ph:          # fully below the diagonal
                        if bt == 0:
                            prev_full[j] = exf       # pair with bt1's tile
                        elif bt == 1:
                            nc.vector.tensor_tensor(af[:], prev_full[j][:],
                                                    exf[:], ADD)
                        else:
                            nc.vector.tensor_tensor(af[:], af[:], exf[:], ADD)
                    else:                            # diagonal pair
                        for cc in range(2):
                            tkc = 2 * bt + cc
                            o = max(0, 128 * tkc - t0)
                            if ph == 0 and bt == 0:
                                if cc == 0:
                                    nc.vector.tensor_copy(a[:, 0], ex[:, 0])
                                else:
                                    nc.vector.memset(a[:, 1, 0:o], 0.0)
                                    nc.vector.tensor_copy(a[:, 1, o:PT],
                                                          ex[:, 1, o:PT])
                            else:
                                nc.vector.tensor_tensor(a[:, cc, o:PT],
                                                        a[:, cc, o:PT],
                                                        ex[:, cc, o:PT], ADD)

                pend = []
                for bt in range(nbt):
                    if bt == 1 and pending_combine is not None:
                        pending_combine()
                        pending_combine = None
                    exs = []
                    for j in range(2):
                        qh, khl = meta[j][0], meta[j][1]
                        ps_s = pps.tile([128, 2, PT], f32,
                                        name=f"pss{ph}_{hl}_{bt}_{j}", tag="s2")
                        for cc in range(2):
                            tkc = 2 * bt + cc
                            o = max(0, 128 * tkc - t0)
                            nc.tensor.matmul(
                                ps_s[:, cc, o:PT],
                                kT[:, khl, 128 * tkc:128 * (tkc + 1)],
                                qTh[:, qh, o:PT], start=True, stop=True)
                        ex = epool.tile([128, 2, PT], f16,
                                        name=f"ex{ph}_{hl}_{bt}_{j}", tag="ex",
                                        bufs=8)
                        o0 = max(0, 128 * 2 * bt - t0)
                        if o0 > 0:  # skip the fully-masked flat prefix
                            psf = ps_s.rearrange("p a b -> p (a b)")
                            exf = ex.rearrange("p a b -> p (a b)")
                            nc.scalar.activation(exf[:, o0:2 * PT],
                                                 psf[:, o0:2 * PT], EXP,
                                                 scale=SCALE)
                        else:
                            nc.scalar.activation(ex[:], ps_s[:], EXP,
                                                 scale=SCALE)
                        for cc in range(2):
                            tkc = 2 * bt + cc
                            o = 128 * tkc - t0
                            if o >= 0:  # diagonal: zero future in 128-wide band
                                nc.gpsimd.affine_select(
                                    ex[:, cc, o:o + 128], ex[:, cc, o:o + 128],
                                    base=0, channel_multiplier=-1,
                                    pattern=[[1, 128]], compare_op=GE, fill=0.0)
                        acc_in(bt, j, ex)
                        exs.append(ex)
                    pend.append((bt, exs))
                    if len(pend) > 3:
                        consume(*pend.pop(0))
                for p in pend:
                    consume(*p)

                if hl == 0:
                    lam_finalize()

                # den: cc-fold on DVE, then ONE all-ones matmul does the
                # cross-partition reduce AND the 128-row broadcast (DVE-side
                # accumulation already collapsed the tk chunks), evacuate,
                # reciprocal on DVE
                rden = []
                for j in range(2):
                    accf = dpool.tile([128, PT], f16, name=f"accf{ph}_{hl}_{j}",
                                      tag="accf")
                    nc.vector.tensor_tensor(accf[:], acc[j][:, 0], acc[j][:, 1],
                                            ADD)
                    ps_rd = pptr.tile([128, PT], f32, name=f"psrd{ph}_{hl}_{j}",
                                      tag="tr")
                    nc.tensor.matmul(ps_rd[:], ones128[:], accf[:],
                                     start=True, stop=True)
                    denb = dpool.tile([128, PT], f32, name=f"denb{ph}_{hl}_{j}",
                                      tag="denb")
                    if j == 0:
                        nc.scalar.copy(denb[:], ps_rd[:])
                    else:
                        nc.vector.tensor_copy(denb[:], ps_rd[:])
                    rd = dpool.tile([128, PT], f32, name=f"rden{ph}_{hl}_{j}",
                                    tag="rden")
                    nc.vector.reciprocal_approx_fast(rd[:], denb[:])
                    rden.append(rd)

                def _combine(hl=hl, ps_y=ps_y, rden=rden):
                    lamT = lam_state["lamT"]
                    ps_lam = pptr.tile([128, PT], f32, name=f"pslam{ph}_{hl}",
                                       tag="tr")
                    for ts in range(4):
                        nc.tensor.matmul(ps_lam[:, 128 * ts:128 * (ts + 1)],
                                         sel16[:, 4 * ts + hl], lamT[:],
                                         start=True, stop=True)
                    m0 = cpool.tile([128, PT], f32, name=f"m0_{ph}_{hl}",
                                    tag="m0")
                    nc.vector.tensor_tensor(m0[:], ps_y[0][:], rden[0][:], MULT)
                    m1 = cpool.tile([128, PT], f32, name=f"m1_{ph}_{hl}",
                                    tag="m1")
                    nc.vector.tensor_tensor(m1[:], ps_y[1][:], rden[1][:], MULT)
                    nc.vector.tensor_tensor(m1[:], m1[:], ps_lam[:], MULT)
                    nc.vector.tensor_tensor(yh[:, hl], m0[:], m1[:], SUB)

                if hl < HPC - 1 and ntk >= 4:
                    pending_combine = _combine
                else:
                    _combine()

            ctx_a.__exit__(None, None, None)
            prev_wo = (ph, yh)
        with nc.named_scope("wo3"):
            emit_wo(*prev_wo, last=True)
    nc.compile()
    return nc


def _get_nc():
    if "nc" not in _CACHE:
        _CACHE["nc"] = _build()
    return _CACHE["nc"]


def kernel(x, Wq1, Wq2, Wk, Wv, Wlam, Wo, **_ignored):
    x = np.ascontiguousarray(np.asarray(x, dtype=np.float32))
    Wq1 = np.asarray(Wq1, dtype=np.float32)
    Wq2 = np.asarray(Wq2, dtype=np.float32)
    Wk = np.asarray(Wk, dtype=np.float32)
    Wv = np.asarray(Wv, dtype=np.float32)
    Wlam = np.asarray(Wlam, dtype=np.float32)
    Wo = np.asarray(Wo, dtype=np.float32)
    F16 = np.float16

    sel16v = np.repeat(np.eye(16, dtype=F16), 128, axis=1)    # [16, 2048]
    idenv = np.eye(128, dtype=np.float32)

    xTs = []
    for b in range(B):
        xt = x[b].T                                   # [D, T]
        xTs.append(np.ascontiguousarray(
            xt.reshape(NDC, 128, NPH, PT).transpose(2, 1, 0, 3).astype(F16)))

    in_maps = []
    for core in range(NC):
        b, g = divmod(core, 4)
        # k/v slots ordered [k(2g), k(8+2g), k(2g+1), k(8+2g+1)] so head-pair
        # hl in {0,1} only needs the first k/v projection pair
        kv_heads = [2 * g, 8 + 2 * g, 2 * g + 1, 8 + 2 * g + 1]
        kv_cols = np.concatenate([np.arange(128 * h, 128 * h + 128)
                                  for h in kv_heads])
        wq_s = np.concatenate([Wq1[:, 512 * g:512 * (g + 1)],
                               Wq2[:, 512 * g:512 * (g + 1)]], axis=1)  # [D, 1024]
        wqp_v = np.ascontiguousarray(
            wq_s.reshape(NDC, 128, 4, 256).transpose(2, 1, 0, 3).astype(F16))
        wk_s = Wk[:, kv_cols]
        wkp_v = np.ascontiguousarray(
            wk_s.reshape(NDC, 128, 2, 256).transpose(2, 1, 0, 3).astype(F16))
        wv_s = Wv[:, kv_cols]                                  # [D, 512]
        wvp_v = np.zeros((2, 128, NDC, 260), dtype=F16)
        for pair in range(2):
            pv = wv_s[:, 256 * pair:256 * (pair + 1)]
            if pair == 0:
                pv = np.concatenate([pv, Wlam[:, 4 * g:4 * (g + 1)]], axis=1)
            wvp_v[pair, :, :, 0:pv.shape[1]] = (
                pv.reshape(NDC, 128, pv.shape[1]).transpose(1, 0, 2).astype(F16))
        wo_s = Wo[512 * g:512 * (g + 1), :]
        wop_v = np.ascontiguousarray(
            wo_s.reshape(HPC, 128, 4, 512).transpose(2, 1, 0, 3).astype(F16))
        in_maps.append({
            "xTp": xTs[b],
            "wqp": wqp_v,
            "wkp": wkp_v,
            "wvp": np.ascontiguousarray(wvp_v),
            "wop": wop_v,
            "sel16in": sel16v,
            "idenin": idenv,
        })

    last_exc = None
    for attempt in range(3):
        try:
            res = run_bass_kernel_spmd(_get_nc(), in_maps, list(range(NC)),
                                       **_CACHE.get("run_kwargs", {}))
            break
        except Exception as e:  # transient NRT device wedges recover on retry
            last_exc = e
            _CACHE.pop("nc", None)
            import time as _time
            _time.sleep(5)
    else:
        raise last_exc
    _CACHE["last_res"] = res
    out = np.zeros((B, T, D), dtype=np.float32)
    for core in range(NC):
        out[core // 4] += res.results[core]["out"].astype(np.float32)
    return out


# revision 55
# speedup vs baseline: 1.0054x; 1.0054x over previous
"""DiffAttnV2-like fused kernel for Trainium2 (8 NeuronCores), v6 (~500us).

Sharding: core = 4*b + g  (b = batch 0..1, g = head-group 0..3, 4 heads each).
Each core computes its 4 output heads' attention and a partial out = y_g @ Wo_g;
host sums the 4 partials per batch.

Changes vs the 587us bf16 baseline (Tensor busy 537->452us):
  - softmax denominator off the PE: exp tiles are accumulated on the DVE
    (f16, 2x mode); then ONE all-ones [128,128] f16 stationary matmul per
    (head, j) does the cross-partition reduce AND the 128-row broadcast in
    512 cycles -> the 139k-cycle ones-matmuls and the 16k den-broadcast
    matmuls are gone. (gpsimd partition_all_reduce was tried: 3.6us/call of
    Pool ucode and it blocks affine_selects - reverted. DVE partition folds
    are impossible: both SBUF inputs must share a base partition.)
  - fp16 storage end-to-end instead of bf16 (same PE rate, 4x less
    quantization error: rel err 5.8e-4 vs 4.2e-3); f16 output partials
  - lam projection rides the v-projection moving weights (4 extra columns,
    psum [*, t, 0:260]); sigmoid computed in natural layout then
    PE-transposed to [16,128] rows and broadcast per head with sel16
    matmuls -> the 33k-cycle dedicated lam matmul chain is gone
  - k/v head slots packed [k(2g), k(8+2g), k(2g+1), k(8+2g+1)] so head
    pairs 0/1 only wait on the first k/v evacuation
  - q projections emitted in order (0,2,1,3) matching attention consumption
  - ppy PSUM pool bufs=3; phase-0 wq/x DMAs split fine-grained so the first
    projection matmul starts ~5us earlier; lam sigmoid chain emitted inside
    attention so it doesn't delay the first exps on the Scalar engine
"""
import sys
sys.path.insert(0, "/opt/trn_rl_repo")
from contextlib import ExitStack

import numpy as np

from concourse import bacc, mybir, tile
from concourse.bass_utils import run_bass_kernel_spmd

B, T, D, H = 2, 2048, 2048, 16
HPC = 4               # heads per core
NC = 8                # cores
NDC = D // 128        # 16 contraction chunks
NPH = 4               # t-phases
PT = T // NPH         # 512 t-cols per phase
SCALE = 1.0 / float(np.sqrt(D // H))

f32 = mybir.dt.float32
f32r = mybir.dt.float32r
f16 = mybir.dt.float16
EXP = mybir.ActivationFunctionType.Exp
GE = mybir.AluOpType.is_ge
ADD = mybir.AluOpType.add
MULT = mybir.AluOpType.mult
SUB = mybir.AluOpType.subtract

_CACHE = {}


def _build():
    nc = bacc.Bacc("TRN2", target_bir_lowering=False, debug=False)
    xTp = nc.dram_tensor("xTp", [NPH, 128, NDC, PT], f16, kind="ExternalInput").ap()
    wqp = nc.dram_tensor("wqp", [4, 128, NDC, 256], f16, kind="ExternalInput").ap()
    wkp = nc.dram_tensor("wkp", [2, 128, NDC, 256], f16, kind="ExternalInput").ap()
    wvp = nc.dram_tensor("wvp", [2, 128, NDC, 260], f16, kind="ExternalInput").ap()
    wop = nc.dram_tensor("wop", [4, 128, HPC, 512], f16, kind="ExternalInput").ap()
    sel16in = nc.dram_tensor("sel16in", [16, 2048], f16, kind="ExternalInput").ap()
    idenin = nc.dram_tensor("idenin", [128, 128], f32, kind="ExternalInput").ap()
    out = nc.dram_tensor("out", [T, D], f16, kind="ExternalOutput").ap()

    with tile.TileContext(nc) as tc, ExitStack() as ctx:
        ctx.enter_context(nc.allow_low_precision(reason="f16/fp32r pipeline"))
        persist = ctx.enter_context(tc.tile_pool(name="persist", bufs=1))
        xpool = ctx.enter_context(tc.tile_pool(name="xpool", bufs=2))
        qpool = ctx.enter_context(tc.tile_pool(name="qpool", bufs=2))
        wpool = ctx.enter_context(tc.tile_pool(name="wpool", bufs=3))
        wvpool = ctx.enter_context(tc.tile_pool(name="wvpool", bufs=2))
        epool = ctx.enter_context(tc.tile_pool(name="epool", bufs=3))
        apool = ctx.enter_context(tc.tile_pool(name="apool", bufs=4))
        dpool = ctx.enter_context(tc.tile_pool(name="dpool", bufs=2))
        cpool = ctx.enter_context(tc.tile_pool(name="cpool", bufs=2))
        ypool = ctx.enter_context(tc.tile_pool(name="ypool", bufs=2))
        opool = ctx.enter_context(tc.tile_pool(name="opool", bufs=4))
        # PSUM banks: pps s2[128,2,512]x2 = 4, ppy [128,512]x3 = 3, pptr = 1
        pps = ctx.enter_context(tc.tile_pool(name="pps", bufs=2, space="PSUM"))
        ppy = ctx.enter_context(tc.tile_pool(name="ppy", bufs=3, space="PSUM"))
        pptr = ctx.enter_context(tc.tile_pool(name="pptr", bufs=1, space="PSUM"))

        kT = persist.tile([128, HPC, T], f16)             # 16KB
        vn = persist.tile([128, 2, NDC, 2, 128], f16)     # 16KB [tk,(pair,tkc,pj),d]
        sel16 = persist.tile([16, 16, 128], f16)          # lam-row selectors
        iden = persist.tile([128, 128], f32)              # transpose identity
        ones128 = persist.tile([128, 128], f16)           # den reduce+bcast
        nc.vector.memset(ones128[:], 1.0)

        def emit_wo(phw, yhw, last=False):
            t0w = PT * phw
            for dout in range(4):
                wo4 = wvpool.tile([128, HPC, 512], f16, name=f"wo{phw}_{dout}",
                                  tag="wo4")
                nc.sync.dma_start(out=wo4[:], in_=wop[dout])
                for tsub in range(4):
                    alt = (dout * 4 + tsub) % 4
                    if alt < 3:
                        ps_o = ppy.tile([128, 512], f32,
                                        name=f"pso{phw}_{dout}_{tsub}", tag="y")
                    else:
                        ps_o = pptr.tile([128, 512], f32,
                                         name=f"pso{phw}_{dout}_{tsub}", tag="tr")
                    for hl in range(HPC):
                        nc.tensor.matmul(
                            ps_o[:], yhw[:, hl, 128 * tsub:128 * (tsub + 1)],
                            wo4[:, hl], start=(hl == 0), stop=(hl == HPC - 1))
                    ob = opool.tile([128, 512], f16,
                                    name=f"ob{phw}_{dout}_{tsub}", tag="ob")
                    vec_mod = 2 if last else 4
                    if (dout * 4 + tsub) % vec_mod == vec_mod - 1:
                        nc.vector.tensor_copy(ob[:], ps_o[:])
                    else:
                        nc.scalar.copy(ob[:], ps_o[:])
                    nc.sync.dma_start(
                        out=out[t0w + 128 * tsub:t0w + 128 * (tsub + 1),
                                512 * dout:512 * (dout + 1)],
                        in_=ob[:])

        prev_wo = None
        for ph in range(NPH):
            t0 = PT * ph
            # ---- phase x^T slice + q weights (ph0: wq halves first for
            #      a fast PE start) ----
            wq_tiles = []
            if ph == 0:
                # fine-grained first loads so the very first matmul chain
                # starts after ~0.4MB instead of ~2MB
                xThA1 = xpool.tile([128, 2, PT], f16, name="xThA1_0",
                                   tag="xThA1", bufs=1)
                nc.sync.dma_start(out=xThA1[:], in_=xTp[ph, :, 0:2])
                wtA1 = wpool.tile([128, 2, 256], f16, name="wq0_0A1",
                                  tag="wqA1", bufs=1)
                nc.sync.dma_start(out=wtA1[:], in_=wqp[0][:, 0:2])
                wtA2 = wpool.tile([128, NDC // 2 - 2, 256], f16, name="wq0_0A2",
                                  tag="wqA2", bufs=1)
                nc.sync.dma_start(out=wtA2[:], in_=wqp[0][:, 2:NDC // 2])
                xThA = xpool.tile([128, NDC // 2 - 2, PT], f16, name=f"xThA{ph}",
                                  tag="xThA", bufs=1)
                nc.sync.dma_start(out=xThA[:], in_=xTp[ph, :, 2:NDC // 2])
                wtB = wpool.tile([128, NDC // 2, 256], f16, name="wq0_0B",
                                 tag="wqB", bufs=1)
                nc.sync.dma_start(out=wtB[:], in_=wqp[0][:, NDC // 2:NDC])
                wq_tiles.append((wtA1, wtA2, wtB))
                xThB = xpool.tile([128, NDC // 2, PT], f16, name=f"xThB{ph}",
                                  tag="xThB")
                nc.sync.dma_start(out=xThB[:], in_=xTp[ph, :, NDC // 2:NDC])

                def xTh_(dc):
                    if dc < 2:
                        return xThA1[:, dc]
                    if dc < NDC // 2:
                        return xThA[:, dc - 2]
                    return xThB[:, dc - NDC // 2]
            else:
                xThA = xpool.tile([128, NDC // 2, PT], f16, name=f"xThA{ph}",
                                  tag="xThAn")
                nc.sync.dma_start(out=xThA[:], in_=xTp[ph, :, 0:NDC // 2])
                xThB = xpool.tile([128, NDC // 2, PT], f16, name=f"xThB{ph}",
                                  tag="xThB")
                nc.sync.dma_start(out=xThB[:], in_=xTp[ph, :, NDC // 2:NDC])

                def xTh_(dc):
                    return (xThA[:, dc] if dc < NDC // 2
                            else xThB[:, dc - NDC // 2])

            # ---- q projections (4 head-pairs) ----
            qTh = qpool.tile([128, 8, PT], f16, name=f"qTh{ph}", tag="qTh")
            ctx_q = nc.named_scope(f"proj{ph}"); ctx_q.__enter__()
            for pq in (0, 2, 1, 3):  # attention heads 0/1 need slots 0,1,4,5
                if ph == 0 and pq == 0:
                    wtA1, wtA2, wtB = wq_tiles[0]

                    def wq_(dc, wtA1=wtA1, wtA2=wtA2, wtB=wtB):
                        if dc < 2:
                            return wtA1[:, dc]
                        if dc < NDC // 2:
                            return wtA2[:, dc - 2]
                        return wtB[:, dc - NDC // 2]
                else:
                    wt = wpool.tile([128, NDC, 256], f16, name=f"wq{ph}_{pq}",
                                    tag="wq")
                    nc.sync.dma_start(out=wt[:], in_=wqp[pq])

                    def wq_(dc, wt=wt):
                        return wt[:, dc]
                ps = pps.tile([128, 2, PT], f32, name=f"psq{ph}_{pq}", tag="s2")
                for j in range(2):
                    for dc in range(NDC):
                        nc.tensor.matmul(ps[:, j],
                                         wq_(dc)[:, 128 * j:128 * (j + 1)],
                                         xTh_(dc),
                                         start=(dc == 0), stop=(dc == NDC - 1))
                for j in range(2):  # per-slot: scores wait one head, not two
                    nc.scalar.copy(qTh[:, 2 * pq + j], ps[:, j])

            # ---- k projections (2 pairs) ----
            for pk in range(2):
                wt = wpool.tile([128, NDC, 256], f16, name=f"wk{ph}_{pk}",
                                tag="wq")
                nc.sync.dma_start(out=wt[:], in_=wkp[pk])
                ps = pps.tile([128, 2, PT], f32, name=f"psk{ph}_{pk}", tag="s2")
                for j in range(2):
                    for dc in range(NDC):
                        nc.tensor.matmul(ps[:, j], wt[:, dc, 128 * j:128 * (j + 1)],
                                         xTh_(dc),
                                         start=(dc == 0), stop=(dc == NDC - 1))
                for j in range(2):  # per-slot k evac
                    nc.vector.tensor_copy(kT[:, 2 * pk + j, t0:t0 + PT],
                                          ps[:, j])

            if ph == 0:  # selector DMAs behind the critical weight loads
                nc.sync.dma_start(out=sel16.rearrange("p a b -> p (a b)"),
                                  in_=sel16in[:])
                nc.sync.dma_start(out=iden[:], in_=idenin[:])

            # ---- v projections (natural [tk, d]); Wlam's 4 cols ride along
            #      pair 0's moving weights producing z = x@Wlam in natural
            #      layout ----
            lamz = cpool.tile([128, 2, 2, HPC], f32, name=f"lamz{ph}",
                              tag="lamz", bufs=1)
            for pair in range(2):
                nvc = 260 if pair == 0 else 256
                wt = wvpool.tile([128, NDC, 260], f16, name=f"wv{ph}_{pair}",
                                 tag="wv")
                nc.sync.dma_start(out=wt[:], in_=wvp[pair])
                for tg in range(2):  # tsub groups of 2
                    ps = pps.tile([128, 2, PT], f32, name=f"psv{ph}_{pair}_{tg}",
                                  tag="s2")
                    for t in range(2):
                        tsub = 2 * tg + t
                        for dc in range(NDC):
                            nc.tensor.matmul(
                                ps[:, t, 0:nvc],
                                xTh_(dc)[:, 128 * tsub:128 * (tsub + 1)],
                                wt[:, dc, 0:nvc],
                                start=(dc == 0), stop=(dc == NDC - 1))
                    if pair == 0:
                        nc.scalar.copy(
                            vn[:, pair, 4 * ph + 2 * tg:4 * ph + 2 * tg + 2],
                            ps[:, :, 0:256])
                        nc.scalar.copy(lamz[:, tg], ps[:, :, 256:260])
                    else:
                        nc.vector.tensor_copy(
                            vn[:, pair, 4 * ph + 2 * tg:4 * ph + 2 * tg + 2],
                            ps[:, :, 0:256])

            # lam = sigmoid(z) chain, emitted later (inside attention at
            # hl==0) so its scalar/DVE ops don't delay the first exps
            lam_state = {}

            def lam_finalize(lamz=lamz, ph=ph):
                lamE = cpool.tile([128, 16], f32, name=f"lamE{ph}", tag="lamE",
                                  bufs=1)
                nc.scalar.activation(lamE[:],
                                     lamz.rearrange("p a b c -> p (a b c)"),
                                     EXP, scale=-1.0)
                nc.vector.tensor_scalar_add(lamE[:], lamE[:], 1.0)
                lamF = cpool.tile([128, 16], f32, name=f"lamF{ph}", tag="lamF",
                                  bufs=1)
                nc.vector.reciprocal_approx_fast(lamF[:], lamE[:])
                psT = pptr.tile([128, PT], f32, name=f"psT{ph}", tag="tr")
                nc.tensor.transpose(psT[0:16, 0:128], lamF[:], iden[:])
                lamT = cpool.tile([16, 128], f16, name=f"lamT{ph}",
                                  tag="lamT", bufs=1)
                nc.scalar.copy(lamT[:], psT[0:16, 0:128])
                lam_state["lamT"] = lamT

            ctx_q.__exit__(None, None, None)
            # ---- Wo of previous phase (fills proj-evac stall window) ----
            if prev_wo is not None:
                with nc.named_scope(f"wo{ph-1}"):
                    emit_wo(*prev_wo)

            # ---- attention: 4 head-pairs ----
            ntk = 4 * (ph + 1)
            nbt = ntk // 2
            yh = ypool.tile([128, HPC, PT], f16, name=f"yh{ph}", tag="yh")
            pending_combine = None
            ctx_a = nc.named_scope(f"attn{ph}"); ctx_a.__enter__()
            for hl in range(HPC):
                meta = []
                for j, qh in ((0, hl), (1, 4 + hl)):
                    khl = 2 * (hl // 2) + j  # k/v slot (host packs [k,k'] pairs)
                    meta.append((qh, khl, khl // 2, khl % 2))
                ps_y = [ppy.tile([128, PT], f32, name=f"psy{ph}_{hl}_{j}",
                                 tag="y") for j in range(2)]
                acc = [apool.tile([128, 2, PT], f16, name=f"acc{ph}_{hl}_{j}",
                                  tag="acc") for j in range(2)]

                def consume(bt, exs):
                    for j in range(2):
                        _, _, pair, pj = meta[j]
                        for cc in range(2):
                            tkc = 2 * bt + cc
                            o = max(0, 128 * tkc - t0)
                            nc.tensor.matmul(ps_y[j][:, o:PT],
                                             vn[:, pair, tkc, pj],
                                             exs[j][:, cc, o:PT],
                                             start=(tkc == 0), stop=(tkc == ntk - 1))

                prev_full = [None, None]

                def acc_in(bt, j, ex):
                    """Fold the exp tile for (bt, j) into acc[j] on the DVE."""
                    a = acc[j]
                    exf = ex.rearrange("p a b -> p (a b)")
                    af = a.rearrange("p a b -> p (a b)")
                    if 2 * bt + 1 < 4 * ph:          # fully below the diagonal
                        if bt == 0:
                            prev_full[j] = exf       # pair with bt1's tile
                        elif bt == 1:
                            nc.vector.tensor_tensor(af[:], prev_full[j][:],
                                                    exf[:], ADD)
                        else:
                            nc.vector.tensor_tensor(af[:], af[:], exf[:], ADD)
                    else:                            # diagonal pair
                        for cc in range(2):
                            tkc = 2 * bt + cc
                            o = max(0, 128 * tkc - t0)
                            if ph == 0 and bt == 0:
                                if cc == 0:
                                    nc.vector.tensor_copy(a[:, 0], ex[:, 0])
                                else:
                                    nc.vector.memset(a[:, 1, 0:o], 0.0)
                                    nc.vector.tensor_copy(a[:, 1, o:PT],
                                                          ex[:, 1, o:PT])
                            else:
                                nc.vector.tensor_tensor(a[:, cc, o:PT],
                                                        a[:, cc, o:PT],
                                                        ex[:, cc, o:PT], ADD)

                pend = []
                for bt in range(nbt):
                    if bt == 1 and pending_combine is not None:
                        pending_combine()
                        pending_combine = None
                    exs = []
                    for j in range(2):
                        qh, khl = meta[j][0], meta[j][1]
                        ps_s = pps.tile([128, 2, PT], f32,
                                        name=f"pss{ph}_{hl}_{bt}_{j}", tag="s2")
                        for cc in range(2):
                            tkc = 2 * bt + cc
                            o = max(0, 128 * tkc - t0)
                            nc.tensor.matmul(
                                ps_s[:, cc, o:PT],
                                kT[:, khl, 128 * tkc:128 * (tkc + 1)],
                                qTh[:, qh, o:PT], start=True, stop=True)
                        ex = epool.tile([128, 2, PT], f16,
                                        name=f"ex{ph}_{hl}_{bt}_{j}", tag="ex",
                                        bufs=8)
                        o0 = max(0, 128 * 2 * bt - t0)
                        if o0 > 0:  # skip the fully-masked flat prefix
                            psf = ps_s.rearrange("p a b -> p (a b)")
                            exf = ex.rearrange("p a b -> p (a b)")
                            nc.scalar.activation(exf[:, o0:2 * PT],
                                                 psf[:, o0:2 * PT], EXP,
                                                 scale=SCALE)
                        else:
                            nc.scalar.activation(ex[:], ps_s[:], EXP,
                                                 scale=SCALE)
                        for cc in range(2):
                            tkc = 2 * bt + cc
                            o = 128 * tkc - t0
                            if o >= 0:  # diagonal: zero future in 128-wide band
                                nc.gpsimd.affine_select(
                                    ex[:, cc, o:o + 128], ex[:, cc, o:o + 128],
                                    base=0, channel_multiplier=-1,
                                    pattern=[[1, 128]], compare_op=GE, fill=0.0)
                        acc_in(bt, j, ex)
                        exs.append(ex)
                    pend.append((bt, exs))
                    if len(pend) > 3:
                        consume(*pend.pop(0))
                for p in pend:
                    consume(*p)

                if hl == 0:
                    lam_finalize()

                # den: cc-fold on DVE, then ONE all-ones matmul does the
                # cross-partition reduce AND the 128-row broadcast (DVE-side
                # accumulation already collapsed the tk chunks), evacuate,
                # reciprocal on DVE
                rden = []
                for j in range(2):
                    accf = dpool.tile([128, PT], f16, name=f"accf{ph}_{hl}_{j}",
                                      tag="accf")
                    nc.vector.tensor_tensor(accf[:], acc[j][:, 0], acc[j][:, 1],
                                            ADD)
                    ps_rd = pptr.tile([128, PT], f32, name=f"psrd{ph}_{hl}_{j}",
                                      tag="tr")
                    nc.tensor.matmul(ps_rd[:], ones128[:], accf[:],
                                     start=True, stop=True)
                    denb = dpool.tile([128, PT], f32, name=f"denb{ph}_{hl}_{j}",
                                      tag="denb")
                    if j == 0:
                        nc.scalar.copy(denb[:], ps_rd[:])
                    else:
                        nc.vector.tensor_copy(denb[:], ps_rd[:])
                    rd = dpool.tile([128, PT], f32, name=f"rden{ph}_{hl}_{j}",
                                    tag="rden")
                    nc.vector.reciprocal_approx_fast(rd[:], denb[:])
                    rden.append(rd)

                def _combine(hl=hl, ps_y=ps_y, rden=rden):
                    lamT = lam_state["lamT"]
                    ps_lam = pptr.tile([128, PT], f32, name=f"pslam{ph}_{hl}",
                                       tag="tr")
                    for ts in range(4):
                        nc.tensor.matmul(ps_lam[:, 128 * ts:128 * (ts + 1)],
                                         sel16[:, 4 * ts + hl], lamT[:],
                                         start=True, stop=True)
                    m0 = cpool.tile([128, PT], f32, name=f"m0_{ph}_{hl}",
                                    tag="m0")
                    nc.vector.tensor_tensor(m0[:], ps_y[0][:], rden[0][:], MULT)
                    m1 = cpool.tile([128, PT], f32, name=f"m1_{ph}_{hl}",
                                    tag="m1")
                    nc.vector.tensor_tensor(m1[:], ps_y[1][:], rden[1][:], MULT)
                    nc.vector.tensor_tensor(m1[:], m1[:], ps_lam[:], MULT)
                    nc.vector.tensor_tensor(yh[:, hl], m0[:], m1[:], SUB)

                if hl < HPC - 1 and ntk >= 4:
                    pending_combine = _combine
                else:
                    _combine()

            ctx_a.__exit__(None, None, None)
            prev_wo = (ph, yh)
        with nc.named_scope("wo3"):
            emit_wo(*prev_wo, last=True)
    nc.compile()
    return nc


def _get_nc():
    if "nc" not in _CACHE:
        _CACHE["nc"] = _build()
    return _CACHE["nc"]


def kernel(x, Wq1, Wq2, Wk, Wv, Wlam, Wo, **_ignored):
    x = np.ascontiguousarray(np.asarray(x, dtype=np.float32))
    Wq1 = np.asarray(Wq1, dtype=np.float32)
    Wq2 = np.asarray(Wq2, dtype=np.float32)
    Wk = np.asarray(Wk, dtype=np.float32)
    Wv = np.asarray(Wv, dtype=np.float32)
    Wlam = np.asarray(Wlam, dtype=np.float32)
    Wo = np.asarray(Wo, dtype=np.float32)
    F16 = np.float16

    sel16v = np.repeat(np.eye(16, dtype=F16), 128, axis=1)    # [16, 2048]
    idenv = np.eye(128, dtype=np.float32)

    xTs = []
    for b in range(B):
        xt = x[b].T                                   # [D, T]
        xTs.append(np.ascontiguousarray(
            xt.reshape(NDC, 128, NPH, PT).transpose(2, 1, 0, 3).astype(F16)))

    in_maps = []
    for core in range(NC):
        b, g = divmod(core, 4)
        # k/v slots ordered [k(2g), k(8+2g), k(2g+1), k(8+2g+1)] so head-pair
        # hl in {0,1} only needs the first k/v projection pair
        kv_heads = [2 * g, 8 + 2 * g, 2 * g + 1, 8 + 2 * g + 1]
        kv_cols = np.concatenate([np.arange(128 * h, 128 * h + 128)
                                  for h in kv_heads])
        wq_s = np.concatenate([Wq1[:, 512 * g:512 * (g + 1)],
                               Wq2[:, 512 * g:512 * (g + 1)]], axis=1)  # [D, 1024]
        wqp_v = np.ascontiguousarray(
            wq_s.reshape(NDC, 128, 4, 256).transpose(2, 1, 0, 3).astype(F16))
        wk_s = Wk[:, kv_cols]
        wkp_v = np.ascontiguousarray(
            wk_s.reshape(NDC, 128, 2, 256).transpose(2, 1, 0, 3).astype(F16))
        wv_s = Wv[:, kv_cols]                                  # [D, 512]
        wvp_v = np.zeros((2, 128, NDC, 260), dtype=F16)
        for pair in range(2):
            pv = wv_s[:, 256 * pair:256 * (pair + 1)]
            if pair == 0:
                pv = np.concatenate([pv, Wlam[:, 4 * g:4 * (g + 1)]], axis=1)
            wvp_v[pair, :, :, 0:pv.shape[1]] = (
                pv.reshape(NDC, 128, pv.shape[1]).transpose(1, 0, 2).astype(F16))
        wo_s = Wo[512 * g:512 * (g + 1), :]
        wop_v = np.ascontiguousarray(
            wo_s.reshape(HPC, 128, 4, 512).transpose(2, 1, 0, 3).astype(F16))
        in_maps.append({
            "xTp": xTs[b],
            "wqp": wqp_v,
            "wkp": wkp_v,
            "wvp": np.ascontiguousarray(wvp_v),
            "wop": wop_v,
            "sel16in": sel16v,
            "idenin": idenv,
        })

    last_exc = None
    for attempt in range(3):
        try:
            res = run_bass_kernel_spmd(_get_nc(), in_maps, list(range(NC)),
                                       **_CACHE.get("run_kwargs", {}))
            break
        except Exception as e:  # transient NRT device wedges recover on retry
            last_exc = e
            _CACHE.pop("nc", None)
            import time as _time
            _time.sleep(5)
    else:
        raise last_exc
    _CACHE["last_res"] = res
    out = np.zeros((B, T, D), dtype=np.float32)
    for core in range(NC):
        out[core // 4] += res.results[core]["out"].astype(np.float32)
    return out
